# revision 9
# baseline (speedup 1.0000x reference)
"""Kernelized (linear) attention on 8 TRN2 NeuronCores.

Strategy (hardcoded for D=2048, H=16, T=4096, B=4, 8 cores):
  - Data-parallel over history T for the heavy K/V projections: core c gets
    t in [c*512, (c+1)*512). Each core computes, for every head/out-channel o
    and batch b:
        ks[o, b] = sum_t relu(k_hist @ Wk.T + bk)[t, b, o]
        kv[o, b] = sum_t (relu(...)+eps) * (v_hist @ Wv.T + bv)[t, b, o]
    via bf16 matmuls in a transposed layout ([out-feature partitions, (b, t)
    free dim]) so biases are per-partition and reductions run along the free
    dim (fused into the producing ops' accum_out).
  - One tiny AllReduce (64 KB) of the [kv | ks] stats across the 8 cores.
  - Every core then redundantly computes the small final stage (q projection,
    combine, Wo projection) and writes the full [4, 2048] output; the host
    takes core 0's result.
"""

import numpy as np
import ml_dtypes

from concourse import bass, bacc, mybir, tile
from concourse.bass_utils import run_bass_kernel_spmd

BF16 = ml_dtypes.bfloat16

D, H, T, B = 2048, 16, 4096, 4
HD = D // H           # 128
NCORES = 8
TLOC = T // NCORES    # 512 history rows per core
R = TLOC * B          # 2048 projection rows per core
NT = D // 128         # 16 tiles along d (contraction) and o (output)
RC = TLOC             # free-dim chunk = one batch element = 512
EPS = 1e-6
F32 = mybir.dt.float32
BF = mybir.dt.bfloat16
AF = mybir.ActivationFunctionType
OP = mybir.AluOpType


def build_nc():
    nc = bacc.Bacc("TRN2", target_bir_lowering=False, debug=False,
                   enable_asserts=False, num_devices=NCORES)

    def din(name, shape, dt):
        return nc.dram_tensor(name, list(shape), dt, kind="ExternalInput").ap()

    xk_d = din("xk", (D, R), BF)            # k shard, [d, b*512+t]
    xv_d = din("xv", (D, R), BF)            # v shard, [d, b*512+t]
    wk_d = din("wk", (NT, 128, D), BF)      # [ot, p(d%128), t(d//128)*128+o_in]
    wv_d = din("wv", (NT, 128, D), BF)
    wq_d = din("wq", (D, D), BF)            # Wq.T  [d, o]
    wo_d = din("wo", (NT, 128, D), BF)      # [ot, p(o_in), o']
    qt_d = din("qt", (128, NT * B), BF)     # [p, t*4+b] = q[b, t*128+p]
    bk_d = din("bk", (128, NT), F32)        # [p, ot]
    bv_d = din("bv", (128, NT), F32)
    bq_d = din("bq", (128, NT), F32)
    bo_d = din("bo", (B, D), F32)           # bo tiled over 4 partitions
    al_d = din("al", (1, H * B), F32)       # alpha repeated x4 (b-fast)
    eye_d = din("eye", (128, 128), F32)
    onc_d = din("onc", (128, 1), F32)       # ones column
    onr_d = din("onr", (1, 128), F32)       # ones row

    out_d = nc.dram_tensor("out", [B, D], F32, kind="ExternalOutput").ap()

    with tile.TileContext(nc) as tc:
        import contextlib
        with contextlib.ExitStack() as ctx:
            p_xk = ctx.enter_context(tc.tile_pool(name="xk", bufs=NT))
            p_xv = ctx.enter_context(tc.tile_pool(name="xv", bufs=NT))
            p_w = ctx.enter_context(tc.tile_pool(name="w", bufs=2))
            p_ep = ctx.enter_context(tc.tile_pool(name="ep", bufs=2))
            p_pr = ctx.enter_context(tc.tile_pool(name="pr", bufs=2))
            p_c1 = ctx.enter_context(tc.tile_pool(name="c1", bufs=1))
            p_qk = ctx.enter_context(tc.tile_pool(name="qk", bufs=NT))
            p_mm = ctx.enter_context(tc.tile_pool(name="mm", bufs=3, space="PSUM"))
            p_a4 = ctx.enter_context(tc.tile_pool(name="a4", bufs=4, space="PSUM"))
            p_tn = ctx.enter_context(tc.tile_pool(name="tn", bufs=1, space="PSUM"))
            p_dr = ctx.enter_context(tc.tile_pool(name="dr", bufs=1, space="DRAM"))

            # ---- resident loads -------------------------------------------
            xk_t, xv_t = [], []
            for t in range(NT):
                a = p_xk.tile([128, R], BF, tag="xk")
                nc.sync.dma_start(out=a[:], in_=xk_d[t * 128:(t + 1) * 128, :])
                xk_t.append(a)
                b_ = p_xv.tile([128, R], BF, tag="xv")
                nc.sync.dma_start(out=b_[:], in_=xv_d[t * 128:(t + 1) * 128, :])
                xv_t.append(b_)

            bk_s = p_c1.tile([128, NT], F32, tag="bk")
            nc.sync.dma_start(out=bk_s[:], in_=bk_d[:, :])
            bv_s = p_c1.tile([128, NT], F32, tag="bv")
            nc.sync.dma_start(out=bv_s[:], in_=bv_d[:, :])
            bq_s = p_c1.tile([128, NT], F32, tag="bq")
            nc.sync.dma_start(out=bq_s[:], in_=bq_d[:, :])
            bo_s = p_c1.tile([B, D], F32, tag="bo")
            nc.sync.dma_start(out=bo_s[:], in_=bo_d[:, :])
            al_s = p_c1.tile([1, H * B], F32, tag="al")
            nc.sync.dma_start(out=al_s[:], in_=al_d[:, :])
            eye_s = p_c1.tile([128, 128], F32, tag="eye")
            nc.sync.dma_start(out=eye_s[:], in_=eye_d[:, :])
            onc_s = p_c1.tile([128, 1], F32, tag="onc")
            nc.sync.dma_start(out=onc_s[:], in_=onc_d[:, :])
            onr_s = p_c1.tile([1, 128], F32, tag="onr")
            nc.sync.dma_start(out=onr_s[:], in_=onr_d[:, :])
            qt_s = p_c1.tile([128, NT * B], BF, tag="qt")
            nc.sync.dma_start(out=qt_s[:], in_=qt_d[:, :])

            # stats staged in one tile: cols [0:64] kv, [64:128] ks
            stat = p_c1.tile([128, 2 * H * B], F32, tag="stat")

            # ---- main loop: K/V projections + fused stats -----------------
            for ot in range(NT):
                wk_s = p_w.tile([128, D], BF, tag="wk")
                nc.sync.dma_start(out=wk_s[:], in_=wk_d[ot])
                wv_s = p_w.tile([128, D], BF, tag="wv")
                nc.sync.dma_start(out=wv_s[:], in_=wv_d[ot])
                for b in range(B):
                    idx = ot * B + b
                    kp = p_mm.tile([128, RC], F32, tag="mm")
                    for t in range(NT):
                        nc.tensor.matmul(
                            kp[:], wk_s[:, t * 128:(t + 1) * 128],
                            xk_t[t][:, b * RC:(b + 1) * RC],
                            start=(t == 0), stop=(t == NT - 1))
                    vp = p_mm.tile([128, RC], F32, tag="mm")
                    for t in range(NT):
                        nc.tensor.matmul(
                            vp[:], wv_s[:, t * 128:(t + 1) * 128],
                            xv_t[t][:, b * RC:(b + 1) * RC],
                            start=(t == 0), stop=(t == NT - 1))
                    kk = p_ep.tile([128, RC], F32, tag="kk")
                    nc.scalar.activation(
                        kk[:], kp[:], AF.Relu, bias=bk_s[:, ot:ot + 1],
                        scale=1.0, accum_out=stat[:, 64 + idx:64 + idx + 1])
                    vb = p_ep.tile([128, RC], F32, tag="vb")
                    nc.vector.tensor_scalar(
                        vb[:], vp[:], bv_s[:, ot:ot + 1], None, OP.add)
                    pr = p_pr.tile([128, RC], BF, tag="pr")
                    nc.vector.scalar_tensor_tensor(
                        pr[:], kk[:], EPS, vb[:], OP.add, OP.mult,
                        accum_out=stat[:, idx:idx + 1])

            # ---- all-reduce the stats across the 8 cores ------------------
            bin_ = p_dr.tile([128, 2 * H * B], F32, tag="bin")
            bout = p_dr.tile([128, 2 * H * B], F32, tag="bout")
            nc.gpsimd.dma_start(out=bin_[:], in_=stat[:])
            nc.gpsimd.collective_compute(
                "AllReduce", OP.add,
                replica_groups=[list(range(NCORES))],
                ins=[bin_.opt()], outs=[bout.opt()])
            ared = p_c1.tile([128, 2 * H * B], F32, tag="ared")
            nc.gpsimd.dma_start(out=ared[:], in_=bout[:])

            # ---- q projection (redundant on every core) -------------------
            qp = [p_a4.tile([B, 512], F32, tag="a4", name=f"qp{i}")
                  for i in range(4)]
            for t in range(NT):
                wq_s = p_w.tile([128, D], BF, tag="wq")
                nc.sync.dma_start(out=wq_s[:], in_=wq_d[t * 128:(t + 1) * 128, :])
                for oc in range(4):
                    nc.tensor.matmul(
                        qp[oc][:], qt_s[:, t * B:(t + 1) * B],
                        wq_s[:, oc * 512:(oc + 1) * 512],
                        start=(t == 0), stop=(t == NT - 1))
            qraw = p_c1.tile([B, D], F32, tag="big4", name="qraw")
            for oc in range(4):
                nc.vector.tensor_copy(qraw[:, oc * 512:(oc + 1) * 512], qp[oc][:])

            # ---- combine stats --------------------------------------------
            # k_sum per head: sum ks over the 128 partitions of each head
            hs = p_tn.tile([1, H * B], F32, tag="tn")
            nc.tensor.matmul(hs[:], onc_s[:], ared[:, 64:128],
                             start=True, stop=True)
            den = p_c1.tile([1, H * B], F32, tag="den")
            # + EPS*T*HD (the +eps inside k_k summed over T*HD) + outer eps
            nc.vector.tensor_scalar(den[:], hs[:], EPS * T * HD + EPS, None,
                                    OP.add)
            rden = p_c1.tile([1, H * B], F32, tag="rden")
            nc.vector.reciprocal(rden[:], den[:])
            rr = p_c1.tile([1, H * B], F32, tag="rr")
            nc.vector.tensor_tensor(rr[:], rden[:], al_s[:], OP.mult)
            # broadcast rr across partitions, fold into kv
            bcr = p_tn.tile([128, H * B], F32, tag="tn")
            nc.tensor.matmul(bcr[:], onr_s[:], rr[:], start=True, stop=True)
            kvr = p_c1.tile([128, H * B], F32, tag="kvr")
            nc.vector.tensor_tensor(kvr[:], ared[:, 0:64], bcr[:], OP.mult)

            # ---- per-head: transpose q_k, combine, accumulate W_o ---------
            op_ps = [p_a4.tile([B, 512], F32, tag="a4", name=f"op{i}")
                     for i in range(4)]
            for ot in range(NT):
                tp = p_tn.tile([128, B], F32, tag="tn")
                nc.tensor.transpose(tp[:], qraw[:, ot * 128:(ot + 1) * 128],
                                    eye_s[:B, :B])
                qkt = p_qk.tile([128, B], F32, tag="qkt")
                nc.vector.tensor_scalar(qkt[:], tp[:], bq_s[:, ot:ot + 1],
                                        0.0, OP.add, OP.max)
                opre = p_qk.tile([128, B], BF, tag="opre")
                nc.vector.scalar_tensor_tensor(
                    opre[:], qkt[:], EPS, kvr[:, ot * B:(ot + 1) * B],
                    OP.add, OP.mult)
                wo_s = p_w.tile([128, D], BF, tag="wo")
                nc.sync.dma_start(out=wo_s[:], in_=wo_d[ot])
                for oc in range(4):
                    nc.tensor.matmul(
                        op_ps[oc][:], opre[:], wo_s[:, oc * 512:(oc + 1) * 512],
                        start=(ot == 0), stop=(ot == NT - 1))

            outf = p_c1.tile([B, D], F32, tag="big4", name="outf")
            for oc in range(4):
                nc.vector.tensor_tensor(
                    outf[:, oc * 512:(oc + 1) * 512], op_ps[oc][:],
                    bo_s[:, oc * 512:(oc + 1) * 512], OP.add)
            nc.sync.dma_start(out=out_d[:, :], in_=outf[:])

    nc.finalize()  # bacc passes incl. alloc_regs()
    # Strip callback/trap pseudo-instructions (they carry virtual registers
    # that walrus's verifier rejects) — same as MultiCoreSim.run_on_hw_raw.
    from concourse import bass_interp
    nc.m = bass_interp.get_hw_module(nc.m)
    return nc


def prep_inputs(q, k_history, v_history, Wq, bq, Wk, bk, Wv, bv, Wo, bo, alpha):
    """Host-side sharding + layout transforms. Returns in_maps for 8 cores."""
    f32 = np.float32

    def wblocks(W):  # [o,d] -> [ot, p(d%128), (d//128)*128 + o_in] bf16
        a = W.astype(f32).reshape(NT, 128, NT, 128)       # (ot, o_in, t, p)
        return np.ascontiguousarray(a.transpose(0, 3, 2, 1)).astype(BF16) \
                 .reshape(NT, 128, D)

    wk = wblocks(Wk)
    wv = wblocks(Wv)
    wq = np.ascontiguousarray(Wq.astype(f32).T).astype(BF16)        # [d, o]
    wo = np.ascontiguousarray(
        Wo.astype(f32).T.reshape(NT, 128, D)).astype(BF16)          # [ot, p, o']
    qt = np.ascontiguousarray(
        q.astype(f32).T.reshape(NT, 128, B).transpose(1, 0, 2)
    ).astype(BF16).reshape(128, NT * B)                             # [p, t*4+b]
    bk_t = np.ascontiguousarray(bk.astype(f32).reshape(NT, 128).T)
    bv_t = np.ascontiguousarray(bv.astype(f32).reshape(NT, 128).T)
    bq_t = np.ascontiguousarray(bq.astype(f32).reshape(NT, 128).T)
    bo_r = np.ascontiguousarray(np.tile(bo.astype(f32)[None, :], (B, 1)))
    al_r = np.ascontiguousarray(
        np.repeat(alpha.astype(f32), B)[None, :])                   # [1, 64]
    eye = np.eye(128, dtype=f32)
    onc = np.ones((128, 1), f32)
    onr = np.ones((1, 128), f32)

    shared = dict(wk=wk, wv=wv, wq=wq, wo=wo, qt=qt, bk=bk_t, bv=bv_t,
                  bq=bq_t, bo=bo_r, al=al_r, eye=eye, onc=onc, onr=onr)

    in_maps = []
    for c in range(NCORES):
        ks_ = k_history[c * TLOC:(c + 1) * TLOC].astype(f32)   # [512, 4, 2048]
        vs_ = v_history[c * TLOC:(c + 1) * TLOC].astype(f32)
        xk = np.ascontiguousarray(ks_.transpose(2, 1, 0).reshape(D, R)) \
               .astype(BF16)
        xv = np.ascontiguousarray(vs_.transpose(2, 1, 0).reshape(D, R)) \
               .astype(BF16)
        in_maps.append(dict(xk=xk, xv=xv, **shared))
    return in_maps


_CACHE = {}


def kernel(**inputs):
    if "nc" not in _CACHE:
        _CACHE["nc"] = build_nc()
    nc = _CACHE["nc"]
    in_maps = prep_inputs(**{k: np.asarray(v) for k, v in inputs.items()})
    res = run_bass_kernel_spmd(nc, in_maps, core_ids=list(range(NCORES)))
    return np.asarray(res.results[0]["out"], dtype=np.float32)
